# revision 32
# baseline (speedup 1.0000x reference)
"""VQ codebook-lookup kernel for Trainium2, data-parallel over 8 NeuronCores.

Problem (nn_BooleanAnchorTable): z [8,4096,256] f32, emb [8192,256] f32.
  dist   = ||z||^2 - 2 z.e + ||e||^2          (fp32, per-op rounding)
  idx    = argmin_j dist (first-occurrence tie-break)
  z_q    = emb[idx];  z_st = z + (z_q - z);  losses = mean((z_q - z)^2)

Sharding: data-parallel over the batch dim — core c owns z[c] (4096 tokens),
codebook replicated; no collectives.  The device computes the 137-GFLOP
distance matmul and the 8192-way argmin per token (idx).  The O(N) tail
(z_q gather, z_st = z + (z_q - z), loss mean) runs in the host unshard step:
numpy fp32 elementwise ops are bitwise-identical to the reference's XLA-CPU
elementwise ops, and the loss reduction is done in float64.

Numerical contract: the grader's fp32 reference quantizes dist at ~ulp(256);
a single argmin flip moves a whole codebook row (~5e-3 rel err on z_st), so
the device reproduces the reference's rounding sequence exactly:
  - PE fp32 matmul yields psum = 2*s bitwise-scaled (z pre-scaled by 2 on the
    host; powers of two commute with fp32 rounding),
  - one fused DVE scalar_tensor_tensor computes (psum - znorm) - enorm with
    per-stage IEEE fp32 rounding = -(fl(fl(znorm-2s)+enorm)) = -dist bitwise,
  - max8 + max_index give the first-occurrence argmax of -dist = jnp.argmin,
  - znorm/enorm come from the same jax-CPU ops the reference uses (bitwise).

Toolchain note: this walrus rejects instructions carrying >1 sync-wait, so the
module is built with bacc.Bacc and nc.compile(), whose passes move matmul
waits onto ldweights and split excess waits into EventSemaphore preludes.
Consts arrive via four region DMAs (parallel queues); quarter-row max8s keep DVE work fine-grained
so PSUM keeps draining and the PE never idles into a HAM re-throttle.
"""

import numpy as np

P = 128          # partitions / tokens per tile
H = 256          # hidden
K = 8192         # codebook size
CH = 512         # codes per psum chunk (fp32 moving-operand max)
N_CHUNKS = K // CH
N_CORES = 8
T_SHARD = 4096   # tokens per core
N_TILES = T_SHARD // P

_BUILT = {}


def _build(n_tiles):
    import concourse.tile as tile
    import concourse.mybir as mybir
    from concourse import bacc
    from contextlib import ExitStack

    f32 = mybir.dt.float32
    u32 = mybir.dt.uint32
    OP = mybir.AluOpType

    # Bacc (not raw Bass): its compile() legalizes sync waits for walrus —
    # moves matmul waits onto ldweights and splits >1-wait instructions into
    # EventSemaphore preludes.
    nc = bacc.Bacc("TRN2", target_bir_lowering=False, debug=False,
                   num_devices=N_CORES)

    t_sh = n_tiles * P
    # One packed const tensor (embT | enorm_bcast | znorm) loaded with a
    # single prologue DMA; z2T (the per-token stationary operands) is
    # streamed per tile so SBUF fits nrow double-buffering.
    ncols = 2 * K + K + n_tiles
    packed = nc.dram_tensor("packed", [P, ncols], f32, kind="ExternalInput").ap()
    z2T = nc.dram_tensor("z2T", [H, t_sh], f32, kind="ExternalInput").ap()
    idxout = nc.dram_tensor("idxout", [P, n_tiles], u32, kind="ExternalOutput").ap()

    # 8 psum chunks of 1024 codes (2 banks each, 4x2=8 banks): wide DVE ops
    # with fine-grained PSUM recycling.
    CW = 1024
    n_cw = K // CW

    with tile.TileContext(nc) as tc:
        with ExitStack() as ctx:
            const = ctx.enter_context(tc.tile_pool(name="const", bufs=1))
            lhsp = ctx.enter_context(tc.tile_pool(name="lhs", bufs=4))
            nrowp = ctx.enter_context(tc.tile_pool(name="nrow", bufs=2))
            psump = ctx.enter_context(tc.tile_pool(name="psum", bufs=4, space="PSUM"))
            topp = ctx.enter_context(tc.tile_pool(name="top8", bufs=2))
            idxp = ctx.enter_context(tc.tile_pool(name="idx8", bufs=2))

            big = const.tile([P, ncols], f32, tag="big")
            # four region DMAs spread the prologue over parallel queues and
            # let the first matmuls start as soon as their embT slice lands
            for a, b_ in ((0, K), (K, 2 * K), (2 * K, 3 * K), (3 * K, ncols)):
                nc.sync.dma_start(out=big[:, a:b_], in_=packed[:, a:b_])
            embT_sb = [big[:, 0:K], big[:, K:2 * K]]
            enorm_b = big[:, 2 * K:3 * K]
            znorm_sb = big[:, 3 * K:3 * K + n_tiles]
            idxall = const.tile([P, n_tiles], u32, tag="idxall")

            for t in range(n_tiles):
                lhs = []
                for kk in range(2):
                    ls = lhsp.tile([P, P], f32, tag=f"lhs{kk}")
                    nc.sync.dma_start(out=ls[:],
                                      in_=z2T[kk * P:(kk + 1) * P, t * P:(t + 1) * P])
                    lhs.append(ls)
                nrow = nrowp.tile([P, K], f32)
                for n in range(n_cw):
                    ps = psump.tile([P, CW], f32)
                    for h in range(CW // CH):
                        c0 = h * CH
                        nc.tensor.matmul(ps[:, c0:c0 + CH], lhsT=lhs[0][:],
                                         rhs=embT_sb[0][:, n * CW + c0:n * CW + c0 + CH],
                                         start=True, stop=False)
                        nc.tensor.matmul(ps[:, c0:c0 + CH], lhsT=lhs[1][:],
                                         rhs=embT_sb[1][:, n * CW + c0:n * CW + c0 + CH],
                                         start=False, stop=True)
                    # -dist chunk = (2s - znorm) - enorm, two IEEE fp32 stages
                    nc.vector.scalar_tensor_tensor(
                        out=nrow[:, n * CW:(n + 1) * CW],
                        in0=ps[:],
                        scalar=znorm_sb[:, t:t + 1],
                        in1=enorm_b[:, n * CW:(n + 1) * CW],
                        op0=OP.subtract,
                        op1=OP.subtract,
                    )

                # quarter-row max8s (finer DVE granularity keeps PSUM draining
                # so PE never idles into a HAM re-throttle), then combine
                q8 = topp.tile([P, 32], f32, tag="q8")
                QW = K // 4
                for q in range(4):
                    nc.vector.max(out=q8[:, q * 8:(q + 1) * 8],
                                  in_=nrow[:, q * QW:(q + 1) * QW])
                top8 = topp.tile([P, 8], f32, tag="top8")
                nc.vector.max(out=top8[:], in_=q8[:])
                idx8 = idxp.tile([P, 8], u32)
                nc.vector.max_index(out=idx8[:], in_max=top8[:], in_values=nrow[:])
                nc.vector.tensor_copy(out=idxall[:, t:t + 1], in_=idx8[:, 0:1])

            nc.sync.dma_start(out=idxout[:, :], in_=idxall[:])

    nc.compile()
    return nc


def _host_norms(z, emb):
    """znorm/enorm with the exact jax-CPU ops the reference uses (bitwise)."""
    import jax
    import jax.numpy as jnp
    cpu = jax.devices("cpu")[0]
    with jax.default_device(cpu):
        flat = jnp.asarray(z, dtype=jnp.float32).reshape(z.shape[0] * z.shape[1], z.shape[2])
        e = jnp.asarray(emb, dtype=jnp.float32)
        znorm = jnp.sum(flat * flat, axis=1, keepdims=True)
        enorm = jnp.sum(e * e, axis=1)
        return np.asarray(znorm).reshape(-1), np.asarray(enorm)


def kernel(z, emb):
    import os
    from concourse.bass_utils import run_bass_kernel_spmd

    z = np.asarray(z, dtype=np.float32)
    emb = np.asarray(emb, dtype=np.float32)
    b, t, h = z.shape
    assert (b, t, h) == (N_CORES, T_SHARD, H) and emb.shape == (K, H)

    znorm_full, enorm = _host_norms(z, emb)
    embT = emb.T                                              # [256, 8192]
    enorm_b = np.broadcast_to(enorm, (P, K))                  # [128, 8192]

    in_maps = []
    for c in range(N_CORES):
        zc = z[c]                                             # [4096, 256]
        z2T = (2.0 * zc).T                                    # [256, 4096] exact
        znc = znorm_full[c * T_SHARD:(c + 1) * T_SHARD]
        znc = znc.reshape(N_TILES, P).T                       # [128, 32]
        packed = np.concatenate(
            [embT[0:P, :], embT[P:2 * P, :], enorm_b, znc], axis=1)
        in_maps.append({"packed": np.ascontiguousarray(packed),
                        "z2T": np.ascontiguousarray(z2T)})

    if "nc" not in _BUILT:
        _BUILT["nc"] = _build(N_TILES)
    nc = _BUILT["nc"]

    import time
    trace = bool(int(os.environ.get("KERNEL_TRACE", "0")))
    t0 = time.time()
    res = run_bass_kernel_spmd(nc, in_maps, core_ids=list(range(N_CORES)),
                               trace=trace)
    _BUILT["last_exec_wall_s"] = time.time() - t0
    _BUILT["last_res"] = res

    idx = np.empty((N_CORES, T_SHARD), dtype=np.int64)
    for c in range(N_CORES):
        idx[c] = res.results[c]["idxout"].T.reshape(-1).astype(np.int64)

    # Host unshard/tail: numpy fp32 elementwise == XLA-CPU elementwise bitwise.
    flat = z.reshape(b * t, h)
    zq = emb[idx.reshape(-1)]
    diff = zq - flat                                   # fp32, one rounding
    z_st = (flat + diff).reshape(b, t, h)              # fp32, one rounding
    sq = diff * diff                                   # fp32, one rounding
    loss = np.float32(sq.astype(np.float64).sum() / sq.size)
    idx32 = idx.astype(np.int32)
    return z_st, idx32, loss, loss
